# revision 21
# baseline (speedup 1.0000x reference)
"""Trainium2 Bass kernel for nn_Discriminator: tensor-parallel over the 32
per-kernel embedding blocks (4 kernels x 512 dims per core, 8 cores).

Per core c (columns cols = c*2048 .. (c+1)*2048 of each W):
  - s2sT / s2iT = (hidden @ W[:, cols])^T computed directly in transposed
    layout [d, row] on the PE (lhsT = W slice, rhs = hidden^T).
  - img_e = fts @ W_img[:, cols] computed in natural layout, then PE-transposed
    to img_eT [d, b].
  - Gram1 (sentence-sentence, per batch, per kernel) via cross-batch-packed
    matmuls: lhsT = s2sT block of one batch-group (8 batches x 16 sents = 128),
    rhs = the 2-batch-group pair (256 cols).  tanh on ScalarE; mask
    (+same-batch, -diagonal, /15) multiply + row-sum on DVE -> dist2sent col.
  - Gram2 (img-sentence) similar with lhsT = img_eT block [128, 16];
    diagonal strip extracted via mask-multiply + ones-vector matmul
    (partition reduction) -> dist2img row.
Matmul operands are fp16 (fp32 accumulate in PSUM): full-rate PE with
overlapped fast weight loads, and half the HBM traffic.
Host: gathers per-core dist slices, applies the tiny (32->2) ff heads.
"""

import os
import sys
from contextlib import ExitStack

import numpy as np

sys.path.insert(0, "/opt/trn_rl_repo")

import concourse.bass as bass
import concourse.bacc as bacc
import concourse.mybir as mybir
from concourse.tile import TileContext
from concourse.bass_utils import run_bass_kernel_spmd

# Problem constants (hardcoded per harness contract)
B, NS, NK, DK = 64, 16, 32, 512
DIM_FT, DIM_HID = 2048, 1024
N_CORES = 8
NKL = NK // N_CORES          # 4 kernels per core
COLS = NKL * DK              # 2048 columns per core
F32 = mybir.dt.float32
INV_SQRT_DK = float(1.0 / np.sqrt(DK))

# matmul operand dtype: fp16 (10-bit mantissa, FWL + overlapped ldweights),
# f32r (single-pass fp32, fused slow weight loads), f32 (exact, 4 cyc/row)
MMDT_NAME = os.environ.get("MMDT", "f16")
MM_DT = {"f16": mybir.dt.float16, "f32r": mybir.dt.float32r,
         "f32": mybir.dt.float32}[MMDT_NAME]
HOST_DT = {"f16": np.float16, "f32r": np.float32, "f32": np.float32}[MMDT_NAME]

KLEVEL = int(os.environ.get("KLEVEL", "9"))


def build_nc() -> bass.Bass:
    nc = bacc.Bacc("TRN2", target_bir_lowering=False, debug=False)

    # hT/fT are host-permuted into SBUF layout: [p, kt, ...] flattened
    hT = nc.dram_tensor("hT", [128, (DIM_HID // 128) * B * NS], MM_DT, kind="ExternalInput")
    fT = nc.dram_tensor("fT", [128, (DIM_FT // 128) * B], MM_DT, kind="ExternalInput")
    w_img = nc.dram_tensor("w_img", [DIM_FT, COLS], MM_DT, kind="ExternalInput")
    w_si = nc.dram_tensor("w_si", [DIM_HID, COLS], MM_DT, kind="ExternalInput")
    w_ss = nc.dram_tensor("w_ss", [DIM_HID, COLS], MM_DT, kind="ExternalInput")
    cmask = nc.dram_tensor("cmask", [128, 512], F32, kind="ExternalInput")
    m2 = nc.dram_tensor("m2", [16, 256], F32, kind="ExternalInput")
    ident = nc.dram_tensor("ident", [64, 64], MM_DT, kind="ExternalInput")
    ones16 = nc.dram_tensor("ones16", [16, 1], MM_DT, kind="ExternalInput")

    d2i = nc.dram_tensor("d2i", [1, 4096], F32, kind="ExternalOutput")
    d2s = nc.dram_tensor("d2s", [128, 32], F32, kind="ExternalOutput")

    KT_H = DIM_HID // 128    # 8 k-tiles for hidden-side matmuls
    KT_F = DIM_FT // 128     # 16 k-tiles for fts-side matmul
    NDC = COLS // 128        # 16 d-chunks per core (4 per kernel)

    with TileContext(nc) as tc, ExitStack() as ctx:
        const = ctx.enter_context(tc.tile_pool(name="const", bufs=1))
        wpool = ctx.enter_context(tc.tile_pool(name="wpool", bufs=2))
        embp = ctx.enter_context(tc.tile_pool(name="embp", bufs=2))
        scr = ctx.enter_context(tc.tile_pool(name="scr", bufs=2))
        pmm = ctx.enter_context(tc.tile_pool(name="pmm", bufs=4, space="PSUM"))
        pgram = ctx.enter_context(tc.tile_pool(name="pgram", bufs=2, space="PSUM"))
        psml = ctx.enter_context(tc.tile_pool(name="psml", bufs=2, space="PSUM"))

        # --- small constants first (cheap), then phase-1 weights stream ---
        fT_sb = const.tile([128, KT_F, B], MM_DT)
        nc.scalar.dma_start(fT_sb[:], fT.rearrange("p (kt b) -> p kt b", kt=KT_F))
        cmask_sb = const.tile([128, 512], F32)
        nc.scalar.dma_start(cmask_sb[:], cmask[:])
        m2_sb = const.tile([16, 256], F32)
        nc.scalar.dma_start(m2_sb[:], m2[:])
        ident_sb = const.tile([64, 64], MM_DT)
        nc.scalar.dma_start(ident_sb[:], ident[:])
        ones_sb = const.tile([16, 1], MM_DT)
        nc.scalar.dma_start(ones_sb[:], ones16[:])

        # --- persistent results ---
        img_eT_sb = const.tile([128, NDC, B], MM_DT)  # [d%128, dchunk, b]
        d2s_sb = const.tile([128, NKL * 8], F32)      # [(b%8)*16+s, k*8+bg]
        d2i_sb = const.tile([1, 4096], F32)           # [(k*4+bgp)*256 + c2]
        if KLEVEL < 9:
            nc.gpsimd.memset(d2s_sb[:], 0.0)
            nc.gpsimd.memset(d2i_sb[:], 0.0)

        w_img_r = w_img.rearrange("(kt p) c -> kt p c", p=128)
        w_si_r = w_si.rearrange("(kt p) c -> p kt c", p=128)
        w_ss_r = w_ss.rearrange("(kt p) c -> p kt c", p=128)
        hT_r = hT.rearrange("p (kt r) -> kt p r", kt=KT_H)

        hT_sb = const.tile([128, KT_H, B * NS], MM_DT)
        if KLEVEL < 1:
            for kth in range(KT_H):
                nc.sync.dma_start(hT_sb[:, kth, :], hT_r[kth, :, :])

        # --- fused loop: per local kernel k, do the img column block cb=k
        # (produces img_eT chunks 4k..4k+4), then the embeddings and Grams.
        # Kernel 0 runs embeddings before the img block (and fetches
        # hT/wss0/wsi0 first) so the PE starts on the earliest-arriving data.
        for k in range(NKL if KLEVEL >= 3 else 0):
            cb = k

            def emit_wdmas():
                wss = wpool.tile([128, KT_H, DK], MM_DT, tag="wss", name="wss")
                nc.sync.dma_start(wss[:], w_ss_r[:, :, k * DK:(k + 1) * DK])
                wsi = wpool.tile([128, KT_H, DK], MM_DT, tag="wsi", name="wsi")
                nc.scalar.dma_start(wsi[:], w_si_r[:, :, k * DK:(k + 1) * DK])
                return wss, wsi

            def emit_img(ps_img=[None]):
                if KLEVEL < 1:
                    return
                ps_img = pmm.tile([128, 512], F32, tag="mm", name="ps_img")[:B]
                for kt in range(KT_F):
                    wt = wpool.tile([128, 512], MM_DT, tag="wimg", bufs=6, name="wimg_t")
                    dma_eng = nc.sync if kt % 2 == 0 else nc.scalar
                    dma_eng.dma_start(wt[:], w_img_r[kt, :, cb * 512:(cb + 1) * 512])
                    nc.tensor.matmul(
                        ps_img, fT_sb[:, kt, :], wt[:],
                        start=(kt == 0), stop=(kt == KT_F - 1),
                    )
                ie = scr.tile([B, 512], MM_DT, tag="imge", name="ie")
                nc.vector.tensor_copy(ie[:], ps_img)
                for dc4 in range(4 if KLEVEL >= 2 else 0):
                    ptr = psml.tile([128, 64], MM_DT, tag="sml", name="ptr")
                    nc.tensor.transpose(ptr[:], ie[:, dc4 * 128:(dc4 + 1) * 128], ident_sb[:])
                    nc.vector.tensor_copy(img_eT_sb[:, cb * 4 + dc4, :], ptr[:])

            def emit_embeds(wss, wsi):
                s2sT = embp.tile([128, 4, B * NS], MM_DT, tag="s2sT", name="s2sT")
                s2iT = embp.tile([128, 4, B * NS], MM_DT, tag="s2iT", name="s2iT")
                for w_sb, dst in ((wsi, s2iT), (wss, s2sT)):
                    for dc in range(4):
                        for rh in range(2):
                            ps = pmm.tile([128, 512], F32, tag="mm", name="ps_emb")
                            for kt in range(KT_H):
                                nc.tensor.matmul(
                                    ps[:],
                                    w_sb[:, kt, dc * 128:(dc + 1) * 128],
                                    hT_sb[:, kt, rh * 512:(rh + 1) * 512],
                                    start=(kt == 0), stop=(kt == KT_H - 1),
                                )
                            nc.vector.tensor_copy(dst[:, dc, rh * 512:(rh + 1) * 512], ps[:])
                return s2sT, s2iT

            if k == 0:
                # chunked first-kernel loads: the first embed matmul only
                # needs wsi[kt=0] + hT[kt=0], so stream both per k-tile
                wsi = wpool.tile([128, KT_H, DK], MM_DT, tag="wsi", name="wsi")
                for kth in range(KT_H):
                    e1 = nc.sync if kth % 2 == 0 else nc.scalar
                    e2 = nc.scalar if kth % 2 == 0 else nc.sync
                    e1.dma_start(hT_sb[:, kth, :], hT_r[kth, :, :])
                    e2.dma_start(wsi[:, kth, :],
                                 w_si_r[:, kth, k * DK:(k + 1) * DK])
                wss = wpool.tile([128, KT_H, DK], MM_DT, tag="wss", name="wss")
                nc.sync.dma_start(wss[:], w_ss_r[:, :, k * DK:(k + 1) * DK])
                s2sT, s2iT = emit_embeds(wss, wsi)
                emit_img()
            else:
                emit_img()
                wss, wsi = emit_wdmas()
                s2sT, s2iT = emit_embeds(wss, wsi)

            # Gram2: img-sentence -> dist2img row (diag extract via mask+ones-mm)
            for bgp in range(4 if KLEVEL >= 6 else 0):
                pg2 = psml.tile([16, 256], F32, tag="sml", name="pg2")
                for dc in range(4):
                    nc.tensor.matmul(
                        pg2[:],
                        img_eT_sb[:, k * 4 + dc, bgp * 16:(bgp + 1) * 16],
                        s2iT[:, dc, bgp * 256:(bgp + 1) * 256],
                        start=(dc == 0), stop=(dc == 3),
                    )
                t2 = scr.tile([16, 256], F32, tag="tanh2", name="t2")
                nc.scalar.activation(t2[:], pg2[:], mybir.ActivationFunctionType.Tanh,
                                     scale=INV_SQRT_DK)
                mk = scr.tile([16, 256], MM_DT, tag="mask2", name="mk")
                nc.vector.tensor_mul(mk[:], t2[:], m2_sb[:])
                if KLEVEL < 7:
                    continue
                pd = psml.tile([1, 256], F32, tag="sml", name="pd")
                nc.tensor.matmul(pd[:], ones_sb[:], mk[:], start=True, stop=True)
                off = (k * 4 + bgp) * 256
                nc.vector.tensor_copy(d2i_sb[:, off:off + 256], pd[:])

            # Gram1: sentence-sentence per batch -> dist2sent column
            for bgl in range(8 if KLEVEL >= 4 else 0):
                pg = pgram.tile([128, 256], F32, tag="g1", name="pg")
                pb = (bgl // 2) * 2 * 128
                for dc in range(4):
                    nc.tensor.matmul(
                        pg[:],
                        s2sT[:, dc, bgl * 128:(bgl + 1) * 128],
                        s2sT[:, dc, pb:pb + 256],
                        start=(dc == 0), stop=(dc == 3),
                    )
                th = scr.tile([128, 256], F32, tag="tanh1", name="th")
                nc.scalar.activation(th[:], pg[:], mybir.ActivationFunctionType.Tanh,
                                     scale=INV_SQRT_DK)
                if KLEVEL < 5:
                    continue
                sc = scr.tile([128, 256], F32, tag="ttr", name="sc")
                col = k * 8 + bgl
                nc.vector.tensor_mul(
                    sc[:], th[:], cmask_sb[:, (bgl % 2) * 256:(bgl % 2) * 256 + 256])
                nc.vector.tensor_reduce(
                    d2s_sb[:, col:col + 1], sc[:],
                    axis=mybir.AxisListType.X, op=mybir.AluOpType.add)

        nc.sync.dma_start(d2s[:], d2s_sb[:])
        nc.sync.dma_start(d2i[:], d2i_sb[:])

    nc.compile()
    return nc


def _host_constants():
    # cmask: [128, 512] = [C_even | C_odd]; rows r=(b_l, s) of the lhs
    # batch-group, cols c2 index the 2-bg-pair (256).
    cm = np.zeros((128, 512), np.float32)
    for r in range(128):
        for c2 in range(256):
            if c2 // 16 == r // 16 and c2 != r:
                cm[r, c2] = 1.0 / (NS - 1)
            if c2 // 16 == 8 + r // 16 and c2 != r + 128:
                cm[r, 256 + c2] = 1.0 / (NS - 1)
    # m2: [16, 256] mask selecting the same-batch strip of Gram2
    m2 = np.zeros((16, 256), np.float32)
    for r in range(16):
        for c2 in range(256):
            if c2 // 16 == r:
                m2[r, c2] = 1.0
    ident = np.eye(64, dtype=HOST_DT)
    ones16 = np.ones((16, 1), HOST_DT)
    return cm, m2, ident, ones16


_NC_CACHE = {}


def kernel(fts, hidden, W_img, W_si, W_ss, ff_img_w, ff_img_b, ff_sent_w, ff_sent_b,
           _trace=False):
    fts = np.asarray(fts, np.float32)
    hidden = np.asarray(hidden, np.float32)
    W_img = np.asarray(W_img, np.float32)
    W_si = np.asarray(W_si, np.float32)
    W_ss = np.asarray(W_ss, np.float32)

    if "nc" not in _NC_CACHE:
        _NC_CACHE["nc"] = build_nc()
    nc = _NC_CACHE["nc"]

    cm, m2, ident, ones16 = _host_constants()
    hT = np.ascontiguousarray(
        hidden.T.astype(HOST_DT).reshape(DIM_HID // 128, 128, B * NS)
        .transpose(1, 0, 2).reshape(128, -1))
    fT = np.ascontiguousarray(
        fts.T.astype(HOST_DT).reshape(DIM_FT // 128, 128, B)
        .transpose(1, 0, 2).reshape(128, -1))
    W_img_h = W_img.astype(HOST_DT)
    W_si_h = W_si.astype(HOST_DT)
    W_ss_h = W_ss.astype(HOST_DT)

    in_maps = []
    for c in range(N_CORES):
        cols = slice(c * COLS, (c + 1) * COLS)
        in_maps.append({
            "hT": hT, "fT": fT,
            "w_img": np.ascontiguousarray(W_img_h[:, cols]),
            "w_si": np.ascontiguousarray(W_si_h[:, cols]),
            "w_ss": np.ascontiguousarray(W_ss_h[:, cols]),
            "cmask": cm, "m2": m2, "ident": ident, "ones16": ones16,
        })

    res = run_bass_kernel_spmd(nc, in_maps, core_ids=list(range(N_CORES)),
                               trace=_trace)
    results = res.results

    d2i_parts, d2s_parts = [], []
    for c in range(N_CORES):
        a = results[c]["d2i"].reshape(NKL, 4, 16, 16)   # [k, bgp, b_loc, s]
        d2i_parts.append(a.transpose(1, 2, 3, 0).reshape(B, NS, NKL))
        b_arr = results[c]["d2s"].reshape(8, 16, NKL, 8)  # [b_l, s, k, bgl]
        d2s_parts.append(b_arr.transpose(3, 0, 1, 2).reshape(B, NS, NKL))

    dist2img = np.concatenate(d2i_parts, axis=2)
    dist2sent = np.concatenate(d2s_parts, axis=2)

    out_img = (dist2img @ ff_img_w + ff_img_b).reshape(B * NS, 2).astype(np.float32)
    out_sent = (dist2sent @ ff_sent_w + ff_sent_b).reshape(B * NS, 2).astype(np.float32)
    out = out_img + out_sent

    if _trace:
        kernel._last_perf = res  # stash for test harness
    return out, out_img, out_sent, dist2img, dist2sent


# revision 22
# speedup vs baseline: 1.0230x; 1.0230x over previous
"""Trainium2 Bass kernel for nn_Discriminator: tensor-parallel over the 32
per-kernel embedding blocks (4 kernels x 512 dims per core, 8 cores).

Per core c (columns cols = c*2048 .. (c+1)*2048 of each W):
  - s2sT / s2iT = (hidden @ W[:, cols])^T computed directly in transposed
    layout [d, row] on the PE (lhsT = W slice, rhs = hidden^T).
  - img_e = fts @ W_img[:, cols] computed in natural layout, then PE-transposed
    to img_eT [d, b].
  - Gram1 (sentence-sentence, per batch, per kernel) via cross-batch-packed
    matmuls: lhsT = s2sT block of one batch-group (8 batches x 16 sents = 128),
    rhs = the 2-batch-group pair (256 cols).  tanh on ScalarE; mask
    (+same-batch, -diagonal, /15) multiply + row-sum on DVE -> dist2sent col.
  - Gram2 (img-sentence) similar with lhsT = img_eT block [128, 16];
    diagonal strip extracted via mask-multiply + ones-vector matmul
    (partition reduction) -> dist2img row.
Matmul operands are fp16 (fp32 accumulate in PSUM): full-rate PE with
overlapped fast weight loads, and half the HBM traffic.
Host: gathers per-core dist slices, applies the tiny (32->2) ff heads.
"""

import os
import sys
from contextlib import ExitStack

import numpy as np

sys.path.insert(0, "/opt/trn_rl_repo")

import concourse.bass as bass
import concourse.bacc as bacc
import concourse.mybir as mybir
from concourse.tile import TileContext
from concourse.bass_utils import run_bass_kernel_spmd

# Problem constants (hardcoded per harness contract)
B, NS, NK, DK = 64, 16, 32, 512
DIM_FT, DIM_HID = 2048, 1024
N_CORES = 8
NKL = NK // N_CORES          # 4 kernels per core
COLS = NKL * DK              # 2048 columns per core
F32 = mybir.dt.float32
INV_SQRT_DK = float(1.0 / np.sqrt(DK))

# matmul operand dtype: fp16 (10-bit mantissa, FWL + overlapped ldweights),
# f32r (single-pass fp32, fused slow weight loads), f32 (exact, 4 cyc/row)
MMDT_NAME = os.environ.get("MMDT", "f16")
MM_DT = {"f16": mybir.dt.float16, "f32r": mybir.dt.float32r,
         "f32": mybir.dt.float32}[MMDT_NAME]
HOST_DT = {"f16": np.float16, "f32r": np.float32, "f32": np.float32}[MMDT_NAME]

KLEVEL = int(os.environ.get("KLEVEL", "9"))


def build_nc() -> bass.Bass:
    nc = bacc.Bacc("TRN2", target_bir_lowering=False, debug=False)

    # hT/fT are host-permuted into SBUF layout: [p, kt, ...] flattened
    hT = nc.dram_tensor("hT", [128, (DIM_HID // 128) * B * NS], MM_DT, kind="ExternalInput")
    fT = nc.dram_tensor("fT", [128, (DIM_FT // 128) * B], MM_DT, kind="ExternalInput")
    w_img = nc.dram_tensor("w_img", [DIM_FT, COLS], MM_DT, kind="ExternalInput")
    w_si = nc.dram_tensor("w_si", [DIM_HID, COLS], MM_DT, kind="ExternalInput")
    w_ss = nc.dram_tensor("w_ss", [DIM_HID, COLS], MM_DT, kind="ExternalInput")
    cmask = nc.dram_tensor("cmask", [128, 512], F32, kind="ExternalInput")
    m2 = nc.dram_tensor("m2", [16, 256], F32, kind="ExternalInput")
    ident = nc.dram_tensor("ident", [64, 64], MM_DT, kind="ExternalInput")
    ones16 = nc.dram_tensor("ones16", [16, 1], MM_DT, kind="ExternalInput")

    d2i = nc.dram_tensor("d2i", [1, 4096], F32, kind="ExternalOutput")
    d2s = nc.dram_tensor("d2s", [128, 32], F32, kind="ExternalOutput")

    KT_H = DIM_HID // 128    # 8 k-tiles for hidden-side matmuls
    KT_F = DIM_FT // 128     # 16 k-tiles for fts-side matmul
    NDC = COLS // 128        # 16 d-chunks per core (4 per kernel)

    with TileContext(nc) as tc, ExitStack() as ctx:
        const = ctx.enter_context(tc.tile_pool(name="const", bufs=1))
        wpool = ctx.enter_context(tc.tile_pool(name="wpool", bufs=2))
        embp = ctx.enter_context(tc.tile_pool(name="embp", bufs=2))
        scr = ctx.enter_context(tc.tile_pool(name="scr", bufs=2))
        pmm = ctx.enter_context(tc.tile_pool(name="pmm", bufs=4, space="PSUM"))
        pgram = ctx.enter_context(tc.tile_pool(name="pgram", bufs=2, space="PSUM"))
        psml = ctx.enter_context(tc.tile_pool(name="psml", bufs=2, space="PSUM"))

        # --- small constants first (cheap), then phase-1 weights stream ---
        fT_sb = const.tile([128, KT_F, B], MM_DT)
        nc.scalar.dma_start(fT_sb[:], fT.rearrange("p (kt b) -> p kt b", kt=KT_F))
        cmask_sb = const.tile([128, 512], F32)
        nc.scalar.dma_start(cmask_sb[:], cmask[:])
        m2_sb = const.tile([16, 256], F32)
        nc.scalar.dma_start(m2_sb[:], m2[:])
        ident_sb = const.tile([64, 64], MM_DT)
        nc.scalar.dma_start(ident_sb[:], ident[:])
        ones_sb = const.tile([16, 1], MM_DT)
        nc.scalar.dma_start(ones_sb[:], ones16[:])

        # --- persistent results ---
        img_eT_sb = const.tile([128, NDC, B], MM_DT)  # [d%128, dchunk, b]
        d2s_sb = const.tile([128, NKL * 8], F32)      # [(b%8)*16+s, k*8+bg]
        d2i_sb = const.tile([1, 4096], F32)           # [(k*4+bgp)*256 + c2]
        if KLEVEL < 9:
            nc.gpsimd.memset(d2s_sb[:], 0.0)
            nc.gpsimd.memset(d2i_sb[:], 0.0)

        w_img_r = w_img.rearrange("(kt p) c -> kt p c", p=128)
        w_si_r = w_si.rearrange("(kt p) c -> p kt c", p=128)
        w_ss_r = w_ss.rearrange("(kt p) c -> p kt c", p=128)
        hT_r = hT.rearrange("p (kt r) -> kt p r", kt=KT_H)

        hT_sb = const.tile([128, KT_H, B * NS], MM_DT)
        if KLEVEL < 1:
            for kth in range(KT_H):
                nc.sync.dma_start(hT_sb[:, kth, :], hT_r[kth, :, :])

        # --- fused loop: per local kernel k, do the img column block cb=k
        # (produces img_eT chunks 4k..4k+4), then the embeddings and Grams.
        # Kernel 0 runs embeddings before the img block (and fetches
        # hT/wss0/wsi0 first) so the PE starts on the earliest-arriving data.
        for k in range(NKL if KLEVEL >= 3 else 0):
            cb = k

            def emit_wdmas():
                wss = wpool.tile([128, KT_H, DK], MM_DT, tag="wss", name="wss")
                nc.sync.dma_start(wss[:], w_ss_r[:, :, k * DK:(k + 1) * DK])
                wsi = wpool.tile([128, KT_H, DK], MM_DT, tag="wsi", name="wsi")
                nc.scalar.dma_start(wsi[:], w_si_r[:, :, k * DK:(k + 1) * DK])
                return wss, wsi

            def emit_img(ps_img=[None]):
                if KLEVEL < 1:
                    return
                ps_img = pmm.tile([128, 512], F32, tag="mm", name="ps_img")[:B]
                for ktg in range(KT_F // 4):
                    wt = wpool.tile([128, 4, 512], MM_DT, tag="wimg", bufs=3, name="wimg_t")
                    dma_eng = nc.sync if ktg % 2 == 0 else nc.scalar
                    dma_eng.dma_start(
                        wt[:],
                        w_img.rearrange("(ktg k4 p) c -> p ktg k4 c", p=128, k4=4)[
                            :, ktg, :, cb * 512:(cb + 1) * 512])
                    for k4 in range(4):
                        kt = ktg * 4 + k4
                        nc.tensor.matmul(
                            ps_img, fT_sb[:, kt, :], wt[:, k4, :],
                            start=(kt == 0), stop=(kt == KT_F - 1),
                        )
                ie = scr.tile([B, 512], MM_DT, tag="imge", name="ie")
                nc.vector.tensor_copy(ie[:], ps_img)
                for dc4 in range(4 if KLEVEL >= 2 else 0):
                    ptr = psml.tile([128, 64], MM_DT, tag="sml", name="ptr")
                    nc.tensor.transpose(ptr[:], ie[:, dc4 * 128:(dc4 + 1) * 128], ident_sb[:])
                    nc.vector.tensor_copy(img_eT_sb[:, cb * 4 + dc4, :], ptr[:])

            def emit_embeds(wss, wsi):
                s2sT = embp.tile([128, 4, B * NS], MM_DT, tag="s2sT", name="s2sT")
                s2iT = embp.tile([128, 4, B * NS], MM_DT, tag="s2iT", name="s2iT")
                for w_sb, dst in ((wsi, s2iT), (wss, s2sT)):
                    for dc in range(4):
                        for rh in range(2):
                            ps = pmm.tile([128, 512], F32, tag="mm", name="ps_emb")
                            for kt in range(KT_H):
                                nc.tensor.matmul(
                                    ps[:],
                                    w_sb[:, kt, dc * 128:(dc + 1) * 128],
                                    hT_sb[:, kt, rh * 512:(rh + 1) * 512],
                                    start=(kt == 0), stop=(kt == KT_H - 1),
                                )
                            nc.vector.tensor_copy(dst[:, dc, rh * 512:(rh + 1) * 512], ps[:])
                return s2sT, s2iT

            if k == 0:
                # big linear loads, split across the two HWDGE queues
                nc.sync.dma_start(hT_sb[:], hT.rearrange("p (kt r) -> p kt r", kt=KT_H))
                wsi = wpool.tile([128, KT_H, DK], MM_DT, tag="wsi", name="wsi")
                nc.scalar.dma_start(wsi[:], w_si_r[:, :, k * DK:(k + 1) * DK])
                wss = wpool.tile([128, KT_H, DK], MM_DT, tag="wss", name="wss")
                nc.sync.dma_start(wss[:], w_ss_r[:, :, k * DK:(k + 1) * DK])
                s2sT, s2iT = emit_embeds(wss, wsi)
                emit_img()
            else:
                emit_img()
                wss, wsi = emit_wdmas()
                s2sT, s2iT = emit_embeds(wss, wsi)

            # Gram2: img-sentence -> dist2img row (diag extract via mask+ones-mm)
            for bgp in range(4 if KLEVEL >= 6 else 0):
                pg2 = psml.tile([16, 256], F32, tag="sml", name="pg2")
                for dc in range(4):
                    nc.tensor.matmul(
                        pg2[:],
                        img_eT_sb[:, k * 4 + dc, bgp * 16:(bgp + 1) * 16],
                        s2iT[:, dc, bgp * 256:(bgp + 1) * 256],
                        start=(dc == 0), stop=(dc == 3),
                    )
                t2 = scr.tile([16, 256], F32, tag="tanh2", name="t2")
                nc.scalar.activation(t2[:], pg2[:], mybir.ActivationFunctionType.Tanh,
                                     scale=INV_SQRT_DK)
                mk = scr.tile([16, 256], MM_DT, tag="mask2", name="mk")
                nc.vector.tensor_mul(mk[:], t2[:], m2_sb[:])
                if KLEVEL < 7:
                    continue
                pd = psml.tile([1, 256], F32, tag="sml", name="pd")
                nc.tensor.matmul(pd[:], ones_sb[:], mk[:], start=True, stop=True)
                off = (k * 4 + bgp) * 256
                nc.vector.tensor_copy(d2i_sb[:, off:off + 256], pd[:])

            # Gram1: sentence-sentence per batch -> dist2sent column
            for bgl in range(8 if KLEVEL >= 4 else 0):
                pg = pgram.tile([128, 256], F32, tag="g1", name="pg")
                pb = (bgl // 2) * 2 * 128
                for dc in range(4):
                    nc.tensor.matmul(
                        pg[:],
                        s2sT[:, dc, bgl * 128:(bgl + 1) * 128],
                        s2sT[:, dc, pb:pb + 256],
                        start=(dc == 0), stop=(dc == 3),
                    )
                th = scr.tile([128, 256], F32, tag="tanh1", name="th")
                nc.scalar.activation(th[:], pg[:], mybir.ActivationFunctionType.Tanh,
                                     scale=INV_SQRT_DK)
                if KLEVEL < 5:
                    continue
                sc = scr.tile([128, 256], F32, tag="ttr", name="sc")
                col = k * 8 + bgl
                nc.vector.tensor_mul(
                    sc[:], th[:], cmask_sb[:, (bgl % 2) * 256:(bgl % 2) * 256 + 256])
                nc.vector.tensor_reduce(
                    d2s_sb[:, col:col + 1], sc[:],
                    axis=mybir.AxisListType.X, op=mybir.AluOpType.add)

        nc.sync.dma_start(d2s[:], d2s_sb[:])
        nc.sync.dma_start(d2i[:], d2i_sb[:])

    nc.compile()
    return nc


def _host_constants():
    # cmask: [128, 512] = [C_even | C_odd]; rows r=(b_l, s) of the lhs
    # batch-group, cols c2 index the 2-bg-pair (256).
    cm = np.zeros((128, 512), np.float32)
    for r in range(128):
        for c2 in range(256):
            if c2 // 16 == r // 16 and c2 != r:
                cm[r, c2] = 1.0 / (NS - 1)
            if c2 // 16 == 8 + r // 16 and c2 != r + 128:
                cm[r, 256 + c2] = 1.0 / (NS - 1)
    # m2: [16, 256] mask selecting the same-batch strip of Gram2
    m2 = np.zeros((16, 256), np.float32)
    for r in range(16):
        for c2 in range(256):
            if c2 // 16 == r:
                m2[r, c2] = 1.0
    ident = np.eye(64, dtype=HOST_DT)
    ones16 = np.ones((16, 1), HOST_DT)
    return cm, m2, ident, ones16


_NC_CACHE = {}


def kernel(fts, hidden, W_img, W_si, W_ss, ff_img_w, ff_img_b, ff_sent_w, ff_sent_b,
           _trace=False):
    fts = np.asarray(fts, np.float32)
    hidden = np.asarray(hidden, np.float32)
    W_img = np.asarray(W_img, np.float32)
    W_si = np.asarray(W_si, np.float32)
    W_ss = np.asarray(W_ss, np.float32)

    if "nc" not in _NC_CACHE:
        _NC_CACHE["nc"] = build_nc()
    nc = _NC_CACHE["nc"]

    cm, m2, ident, ones16 = _host_constants()
    hT = np.ascontiguousarray(
        hidden.T.astype(HOST_DT).reshape(DIM_HID // 128, 128, B * NS)
        .transpose(1, 0, 2).reshape(128, -1))
    fT = np.ascontiguousarray(
        fts.T.astype(HOST_DT).reshape(DIM_FT // 128, 128, B)
        .transpose(1, 0, 2).reshape(128, -1))
    W_img_h = W_img.astype(HOST_DT)
    W_si_h = W_si.astype(HOST_DT)
    W_ss_h = W_ss.astype(HOST_DT)

    in_maps = []
    for c in range(N_CORES):
        cols = slice(c * COLS, (c + 1) * COLS)
        in_maps.append({
            "hT": hT, "fT": fT,
            "w_img": np.ascontiguousarray(W_img_h[:, cols]),
            "w_si": np.ascontiguousarray(W_si_h[:, cols]),
            "w_ss": np.ascontiguousarray(W_ss_h[:, cols]),
            "cmask": cm, "m2": m2, "ident": ident, "ones16": ones16,
        })

    res = run_bass_kernel_spmd(nc, in_maps, core_ids=list(range(N_CORES)),
                               trace=_trace)
    results = res.results

    d2i_parts, d2s_parts = [], []
    for c in range(N_CORES):
        a = results[c]["d2i"].reshape(NKL, 4, 16, 16)   # [k, bgp, b_loc, s]
        d2i_parts.append(a.transpose(1, 2, 3, 0).reshape(B, NS, NKL))
        b_arr = results[c]["d2s"].reshape(8, 16, NKL, 8)  # [b_l, s, k, bgl]
        d2s_parts.append(b_arr.transpose(3, 0, 1, 2).reshape(B, NS, NKL))

    dist2img = np.concatenate(d2i_parts, axis=2)
    dist2sent = np.concatenate(d2s_parts, axis=2)

    out_img = (dist2img @ ff_img_w + ff_img_b).reshape(B * NS, 2).astype(np.float32)
    out_sent = (dist2sent @ ff_sent_w + ff_sent_b).reshape(B * NS, 2).astype(np.float32)
    out = out_img + out_sent

    if _trace:
        kernel._last_perf = res  # stash for test harness
    return out, out_img, out_sent, dist2img, dist2sent
